# revision 2
# baseline (speedup 1.0000x reference)
"""Trainium2 Bass kernel for nn_DRNLayer (8-core batch-sharded, Chebyshev).

Math: out[i,j,l] = softmax_l( sum_k phi_ikl(w[j,k]) + B[j,l] ) where
  phi_ikl(w) = ln( sum_m exp(w*logD[l,m]) * P[i,k,m] )
is analytic in the scalar w (a cumulant generating function).  Over the
weight range [-wmax, wmax] (wmax ~ 0.4) its Chebyshev interpolant of
degree S-1 is accurate to ~1e-6 at S=5, so

  logsum[i,j,l] ~= sum_s Lam_s[j,k-summed] :  sum_{s,k} Lam_s[j,k] * phi_s[i,k,l]

with Lam_s[j,k] = Lagrange cardinal poly l_s(w[j,k]) computed on host.
This removes the 64M-element T/Pw tensors of the direct algorithm: the
device only computes S small moment matmuls, S*|ikl| logs, and an
S-term contraction over k.

Sharding: data-parallel over batch i (8 per core); all parameters are
replicated.  No collectives; host concatenates per-core outputs.

Per-core dataflow:
  - step1: per i, one matmul m_s[k, (s,l)] = P_i^T[m,k].T @ DS[m,(s,l)]
    (K=64, M=128, N=S*64) -> one PSUM bank per i (8 banks).
  - Ln on ACT: PSUM -> SBUF phi[k, i, (s,l)].
  - step3: S accumulating matmuls logits[j,(i,l)] += LAM_s[k,j].T @ phi_s
    (K=128, M=128, N=512) -> PSUM.
  - bias add + softmax over l on DVE/ACT, DMA out [j, i, l].
"""

import sys

sys.path.insert(0, "/opt/trn_rl_repo")

from contextlib import ExitStack

import numpy as np

import concourse.bacc as bacc
import concourse.bass as bass
import concourse.mybir as mybir
from concourse.bass_utils import run_bass_kernel_spmd
from concourse.tile import TileContext

F32 = mybir.dt.float32
F32R = mybir.dt.float32r
AF = mybir.ActivationFunctionType
ALU = mybir.AluOpType

N_CORES = 8
BATCH = 64
IB = BATCH // N_CORES  # 8 batch rows per core
NJ = 128  # n_upper (all j on every core)
NK = 128  # n_lower
Q = 64  # q_upper == q_lower
S = 5  # Chebyshev nodes
ACT_TABLE_LN_EXP = 6  # act_info.json index of natural_log_exp_and_others

_NC = None
LAST_RESULTS = None


def _build():
    nc = bacc.Bacc()
    P_d = nc.declare_dram_parameter("PT", [Q, IB, NK], F32R, isOutput=False)
    D_d = nc.declare_dram_parameter("DS", [Q, S * Q], F32R, isOutput=False)
    L_d = nc.declare_dram_parameter("LAM", [NK, S, NJ], F32R, isOutput=False)
    B_d = nc.declare_dram_parameter("BB", [NJ, Q], F32, isOutput=False)
    o_d = nc.declare_dram_parameter("out", [NJ, IB, Q], F32, isOutput=True)

    with TileContext(nc) as tc, ExitStack() as ctx:
        # keep Exp+Ln resident in one ACT table for the whole kernel
        nc.scalar.add_instruction(
            mybir.InstLoadActFuncSet(
                name=nc.get_next_instruction_name(),
                ins=[],
                outs=[],
                act_func_set_id=ACT_TABLE_LN_EXP,
            )
        )

        consts = ctx.enter_context(tc.tile_pool(name="consts", bufs=1))
        phip = ctx.enter_context(tc.tile_pool(name="phi", bufs=1))
        spool = ctx.enter_context(tc.tile_pool(name="smax", bufs=1))

        DSt = consts.tile([Q, S * Q], F32R)
        nc.sync.dma_start(out=DSt, in_=D_d[:, :])
        PT = consts.tile([Q, IB, NK], F32R)
        nc.sync.dma_start(out=PT, in_=P_d[:, :, :])
        LAMt = consts.tile([NK, S, NJ], F32R)
        nc.gpsimd.dma_start(out=LAMt, in_=L_d[:, :, :])
        BBt = consts.tile([NJ, Q], F32)
        nc.gpsimd.dma_start(out=BBt, in_=B_d[:, :])

        PHI = phip.tile([NK, IB, S * Q], F32R)

        # ---------------- step 1: moments + Ln ----------------
        with tc.tile_pool(name="ps1", bufs=1, space="PSUM") as ps1p:
            ps1 = ps1p.tile([NK, IB, 512], F32)
            for i in range(IB):
                nc.tensor.matmul(
                    out=ps1[:, i, 0 : S * Q],
                    lhsT=PT[:, i, :],
                    rhs=DSt,
                    start=True,
                    stop=True,
                )
                # Ln each PSUM bank as soon as it is written
                nc.scalar.activation(
                    out=PHI[:, i, :],
                    in_=ps1[:, i, 0 : S * Q],
                    func=AF.Ln,
                )

        # ---------------- step 3: contraction over (s, k) ----------------
        with tc.tile_pool(name="lg", bufs=1, space="PSUM") as ps2p:
            lg = ps2p.tile([NJ, IB, Q], F32)
            for s in range(S):
                nc.tensor.matmul(
                    out=lg.rearrange("p a b -> p (a b)"),
                    lhsT=LAMt[:, s, :],
                    rhs=PHI[:, :, s * Q : (s + 1) * Q],
                    start=(s == 0),
                    stop=(s == S - 1),
                    skip_group_check=True,
                )

            # ---------------- bias + softmax over l ----------------
            em = spool.tile([NJ, IB, Q], F32)
            nc.vector.tensor_tensor(
                out=em,
                in0=lg,
                in1=BBt.unsqueeze(1).broadcast_to([NJ, IB, Q]),
                op=ALU.add,
            )
            mx = spool.tile([NJ, IB], F32)
            nc.vector.tensor_reduce(mx, em, axis=mybir.AxisListType.X, op=ALU.max)
            nc.vector.tensor_tensor(
                out=em,
                in0=em,
                in1=mx.unsqueeze(2).broadcast_to([NJ, IB, Q]),
                op=ALU.subtract,
            )
            nc.scalar.activation(out=em, in_=em, func=AF.Exp)
            sm = spool.tile([NJ, IB], F32)
            nc.vector.tensor_reduce(sm, em, axis=mybir.AxisListType.X, op=ALU.add)
            rec = spool.tile([NJ, IB], F32)
            nc.vector.reciprocal(rec, sm)
            oute = spool.tile([NJ, IB, Q], F32)
            nc.vector.tensor_tensor(
                out=oute,
                in0=em,
                in1=rec.unsqueeze(2).broadcast_to([NJ, IB, Q]),
                op=ALU.mult,
            )
            nc.sync.dma_start(out=o_d[:, :, :], in_=oute)

    nc.compile()
    return nc


def kernel(P, weight, bias_abs, bias_q, lambda_abs, lambda_q):
    global _NC, LAST_RESULTS
    P = np.asarray(P, dtype=np.float32)
    weight = np.asarray(weight, dtype=np.float32)
    bias_abs = np.asarray(bias_abs, dtype=np.float32)
    bias_q = np.asarray(bias_q, dtype=np.float32)
    lambda_abs = np.asarray(lambda_abs, dtype=np.float32)
    lambda_q = np.asarray(lambda_q, dtype=np.float32)

    if _NC is None:
        _NC = _build()

    qv = np.arange(Q, dtype=np.float32) / Q
    logD = -(qv[None, :] - qv[:, None]) ** 2  # [l, m]

    wmax = float(np.abs(weight).max())
    if wmax == 0.0:
        wmax = 1e-6
    ws = np.cos((2 * np.arange(S) + 1) / (2 * S) * np.pi) * wmax  # [S]

    Ds = np.exp(ws[:, None, None] * logD[None, :, :])  # [S, l, m]
    DS = np.ascontiguousarray(
        Ds.transpose(2, 0, 1).reshape(Q, S * Q).astype(np.float32)
    )  # [m, (s,l)]

    Lam = np.ones((S,) + weight.shape, dtype=np.float64)  # [S, j, k]
    for s in range(S):
        for r in range(S):
            if r != s:
                Lam[s] *= (weight - ws[r]) / (ws[s] - ws[r])
    LAM = np.ascontiguousarray(
        Lam.transpose(2, 0, 1).astype(np.float32)
    )  # [k, s, j]

    sv = qv[None, :]  # [1, 64]
    B = (-bias_q * (sv - lambda_q) ** 2 - bias_abs * np.abs(sv - lambda_abs)).astype(
        np.float32
    )  # [j, l]

    PTfull = np.ascontiguousarray(P.transpose(2, 0, 1))  # [m, i, k]

    in_maps = []
    for c in range(N_CORES):
        in_maps.append(
            {
                "PT": np.ascontiguousarray(PTfull[:, c * IB : (c + 1) * IB, :]),
                "DS": DS,
                "LAM": LAM,
                "BB": B,
            }
        )

    LAST_RESULTS = run_bass_kernel_spmd(_NC, in_maps, list(range(N_CORES)))
    return np.concatenate(
        [LAST_RESULTS.results[c]["out"].transpose(1, 0, 2) for c in range(N_CORES)],
        axis=0,
    )


# revision 9
# speedup vs baseline: 1.0391x; 1.0391x over previous
"""Trainium2 Bass kernel for nn_DRNLayer (8-core batch-sharded, Chebyshev).

Math: out[i,j,l] = softmax_l( sum_k phi_ikl(w[j,k]) + B[j,l] ) where
  phi_ikl(w) = ln( sum_m exp(w*logD[l,m]) * P[i,k,m] )
is analytic in the scalar w (a cumulant generating function).  Over the
weight range [-wmax, wmax] (wmax ~ 0.4) its Chebyshev interpolant of
degree S-1 is accurate to ~1e-4 at S=4, so

  logsum[i,j,l] ~= sum_{s,k} Lam_s[j,k] * phi_s[i,k,l]

with Lam_s[j,k] = Lagrange cardinal poly l_s(w[j,k]) computed on host.
This removes the 64M-element T/Pw tensors of the direct algorithm: the
device only computes S small moment matmuls, S*|ikl| logs, and an
S-term contraction over k.

Sharding: data-parallel over batch i (8 per core); all parameters are
replicated.  No collectives; host concatenates per-core outputs.

Per-core dataflow:
  - warmup matmuls during the input DMA wait keep the PE HAM un-throttled.
  - step1: 4 passes, each computing two i at once via row-tiled half-array
    matmuls (even i on partitions 0-63, odd i on 64-127), K=64, M=128,
    N=S*64 -> one PSUM bank per i.
  - Ln on ACT per bank pair: PSUM -> SBUF phi[k, i, (s,l)].
  - step3 per i-half: S accumulating matmuls logits[j,(i,l)] += LAM_s.T @
    phi_s plus one bias matmul B^T.T @ eye (K=l-identity) -> PSUM.
  - softmax over l per i-half on DVE/ACT/Pool, DMA out [j, i, l] on two
    queues.
"""

import sys

sys.path.insert(0, "/opt/trn_rl_repo")

from contextlib import ExitStack

import numpy as np

import concourse.bacc as bacc
import concourse.bass as bass
import concourse.mybir as mybir
from concourse.bass_utils import run_bass_kernel_spmd
from concourse.tile import TileContext

F32 = mybir.dt.float32
F32R = mybir.dt.float32r
I32 = mybir.dt.int32
AF = mybir.ActivationFunctionType
ALU = mybir.AluOpType

N_CORES = 8
BATCH = 64
IB = BATCH // N_CORES  # 8 batch rows per core
IH = IB // 2  # half for tail pipelining
NJ = 128  # n_upper (all j on every core)
NK = 128  # n_lower
Q = 64  # q_upper == q_lower
S = 4  # Chebyshev nodes
SQ = S * Q
N_WARM = 5
ACT_TABLE_LN_EXP = 6  # act_info.json index of natural_log_exp_and_others

_NC = None
LAST_RESULTS = None


def _build():
    nc = bacc.Bacc()
    # PT2[m + 64*(i%2), i//2, k] = P[i, k, m]
    P_d = nc.declare_dram_parameter("PT2", [128, IB // 2, NK], F32R, isOutput=False)
    # DS2[m + 64*c, (s,l)] = exp(ws[s]*logD[l,m]) (same for both halves c)
    D_d = nc.declare_dram_parameter("DS2", [128, SQ], F32R, isOutput=False)
    L_d = nc.declare_dram_parameter("LAM", [NK, S, NJ], F32R, isOutput=False)
    B_d = nc.declare_dram_parameter("BT", [Q, NJ], F32R, isOutput=False)
    o_d = nc.declare_dram_parameter("out", [NJ, IB, Q], F32, isOutput=True)

    with TileContext(nc) as tc, ExitStack() as ctx:
        consts = ctx.enter_context(tc.tile_pool(name="consts", bufs=1))
        phip = ctx.enter_context(tc.tile_pool(name="phi", bufs=1))
        spool = ctx.enter_context(tc.tile_pool(name="smax", bufs=1))

        # ---------------- input DMAs (parallel queues) ----------------
        PT2 = consts.tile([128, IB // 2, NK], F32R)
        nc.sync.dma_start(out=PT2, in_=P_d[:, :, :])
        DS2 = consts.tile([128, SQ], F32R)
        nc.scalar.dma_start(out=DS2, in_=D_d[:, :])
        LAMt = consts.tile([NK, S, NJ], F32R)
        nc.gpsimd.dma_start(out=LAMt, in_=L_d[:, :, :])
        BTt = consts.tile([Q, NJ], F32R)
        nc.gpsimd.dma_start(out=BTt, in_=B_d[:, :])

        # keep Exp+Ln resident in one ACT table for the whole kernel
        nc.scalar.add_instruction(
            mybir.InstLoadActFuncSet(
                name=nc.get_next_instruction_name(),
                ins=[],
                outs=[],
                act_func_set_id=ACT_TABLE_LN_EXP,
            )
        )

        # ---------------- on-device constants ----------------
        # eye8[p, i, l] = (l == p), p < 64
        it = consts.tile([Q, Q], I32)
        nc.gpsimd.iota(it, pattern=[[1, Q]], base=0, channel_multiplier=-1)
        eye = consts.tile([Q, Q], F32)
        nc.gpsimd.tensor_scalar(eye, it, 0, None, ALU.is_equal)
        eye8 = consts.tile([Q, IB, Q], F32R)
        nc.vector.tensor_copy(
            out=eye8, in_=eye.unsqueeze(1).broadcast_to([Q, IB, Q])
        )
        eyeflat = eye8.rearrange("p a b -> p (a b)")

        PHI = phip.tile([NK, IB, SQ], F32R)

        with tc.tile_pool(name="ps1", bufs=1, space="PSUM") as ps1p:
            ps1 = ps1p.tile([NK, IB, 512], F32)

            # ---------------- PE warmup during DMA wait ----------------
            for _ in range(N_WARM):
                nc.tensor.matmul(
                    out=ps1[0:Q, IB - 1, :],
                    lhsT=eye8[:, 0, :],
                    rhs=eyeflat,
                    start=True,
                    stop=True,
                    skip_group_check=True,
                )

            # ---------------- step 1: moments + Ln ----------------
            for p in range(IB // 2):
                for c in range(2):  # row-tiled half-array matmuls, 2 i at once
                    nc.tensor.matmul(
                        out=ps1[:, 2 * p + c, 0:SQ],
                        lhsT=PT2[64 * c : 64 * (c + 1), p, :],
                        rhs=DS2[64 * c : 64 * (c + 1), :],
                        start=True,
                        stop=True,
                        skip_group_check=True,
                    )
                nc.scalar.activation(
                    out=PHI[:, 2 * p : 2 * p + 2, :],
                    in_=ps1[:, 2 * p : 2 * p + 2, 0:SQ],
                    func=AF.Ln,
                )

        # ---------------- step 3 + softmax, per i-half ----------------
        with tc.tile_pool(name="lg", bufs=1, space="PSUM") as ps2p:
            lg = ps2p.tile([NJ, IB, Q], F32)
            for h in range(2):
                isl = slice(h * IH, (h + 1) * IH)
                out_h = lg[:, isl, :].rearrange("p a b -> p (a b)")
                for s in range(S):
                    nc.tensor.matmul(
                        out=out_h,
                        lhsT=LAMt[:, s, :],
                        rhs=PHI[:, isl, s * Q : (s + 1) * Q],
                        start=(s == 0),
                        stop=False,
                        skip_group_check=True,
                    )
                nc.tensor.matmul(
                    out=out_h,
                    lhsT=BTt,
                    rhs=eye8[:, isl, :],
                    start=False,
                    stop=True,
                    skip_group_check=True,
                )

                lgh = lg[:, isl, :]
                mx = spool.tile([NJ, IH], F32, tag=f"mx{h}")
                nc.vector.tensor_reduce(mx, lgh, axis=mybir.AxisListType.X, op=ALU.max)
                em = spool.tile([NJ, IH, Q], F32, tag=f"em{h}")
                nc.vector.tensor_tensor(
                    out=em,
                    in0=lgh,
                    in1=mx.unsqueeze(2).broadcast_to([NJ, IH, Q]),
                    op=ALU.subtract,
                )
                nc.scalar.activation(out=em, in_=em, func=AF.Exp)
                sm = spool.tile([NJ, IH], F32, tag=f"sm{h}")
                nc.vector.tensor_reduce(sm, em, axis=mybir.AxisListType.X, op=ALU.add)
                rec = spool.tile([NJ, IH], F32, tag=f"rec{h}")
                nc.vector.reciprocal(rec, sm)
                oute = spool.tile([NJ, IH, Q], F32, tag=f"oute{h}")
                eng = nc.gpsimd if h == 0 else nc.vector
                eng.tensor_tensor(
                    out=oute,
                    in0=em,
                    in1=rec.unsqueeze(2).broadcast_to([NJ, IH, Q]),
                    op=ALU.mult,
                )
                qeng = nc.gpsimd if h == 0 else nc.sync
                qeng.dma_start(out=o_d[:, isl, :], in_=oute)

    nc.compile()
    return nc


def kernel(P, weight, bias_abs, bias_q, lambda_abs, lambda_q):
    global _NC, LAST_RESULTS
    P = np.asarray(P, dtype=np.float32)
    weight = np.asarray(weight, dtype=np.float32)
    bias_abs = np.asarray(bias_abs, dtype=np.float32)
    bias_q = np.asarray(bias_q, dtype=np.float32)
    lambda_abs = np.asarray(lambda_abs, dtype=np.float32)
    lambda_q = np.asarray(lambda_q, dtype=np.float32)

    if _NC is None:
        _NC = _build()

    qv = np.arange(Q, dtype=np.float32) / Q
    logD = -(qv[None, :] - qv[:, None]) ** 2  # [l, m]

    wmax = float(np.abs(weight).max())
    if wmax == 0.0:
        wmax = 1e-6
    ws = np.cos((2 * np.arange(S) + 1) / (2 * S) * np.pi) * wmax  # [S]

    Ds = np.exp(ws[:, None, None] * logD[None, :, :])  # [S, l, m]
    DS = Ds.transpose(2, 0, 1).reshape(Q, SQ).astype(np.float32)  # [m, (s,l)]
    DS2 = np.ascontiguousarray(np.concatenate([DS, DS], axis=0))  # [128, SQ]

    Lam = np.ones((S,) + weight.shape, dtype=np.float64)  # [S, j, k]
    for s in range(S):
        for r in range(S):
            if r != s:
                Lam[s] *= (weight - ws[r]) / (ws[s] - ws[r])
    LAM = np.ascontiguousarray(Lam.transpose(2, 0, 1).astype(np.float32))  # [k, s, j]

    sv = qv[None, :]  # [1, 64]
    B = (-bias_q * (sv - lambda_q) ** 2 - bias_abs * np.abs(sv - lambda_abs)).astype(
        np.float32
    )  # [j, l]
    BT = np.ascontiguousarray(B.T)  # [l, j]

    # PT2[m + 64*(i%2), i//2, k] = P[i, k, m] per core slice
    PTfull = P.transpose(2, 0, 1)  # [m, i, k]

    in_maps = []
    for c in range(N_CORES):
        sl = PTfull[:, c * IB : (c + 1) * IB, :]  # [64, 8, 128]
        PT2 = np.empty((128, IB // 2, NK), dtype=np.float32)
        PT2[0:64] = sl[:, 0::2, :]
        PT2[64:128] = sl[:, 1::2, :]
        in_maps.append(
            {
                "PT2": np.ascontiguousarray(PT2),
                "DS2": DS2,
                "LAM": LAM,
                "BT": BT,
            }
        )

    LAST_RESULTS = run_bass_kernel_spmd(_NC, in_maps, list(range(N_CORES)))
    out = np.empty((BATCH, NJ, Q), dtype=np.float32)
    for c in range(N_CORES):
        o = LAST_RESULTS.results[c]["out"]  # [j, ib, l]
        ib = o.shape[1]
        # interleaved: psum bank 2p+c holds i = 2p + c -> matches [j, i, l]
        out[c * IB : (c + 1) * IB] = o.transpose(1, 0, 2)
    return out
